# revision 15
# baseline (speedup 1.0000x reference)
"""Dcls2d (dilated conv with learnable spacings) on 8 Trainium2 NeuronCores.

Math: kern[o,c,h,w] = sum_k weight[o,c,k] * hat(ph[c,k]-h) * hat(pw[c,k]-w)
      (hat(t) = relu(1-|t|) reproduces the reference's bilinear corner fracs
      bit-exactly), then out = conv2d(x, kern, pad=3) + bias.

Sharding: data-parallel over batch — 4 images per core, weight/P/bias
replicated; the (tiny) kernel construction is redone on every core on the
vector engine, the conv runs on the tensor engine as 49 PSUM-accumulated
matmuls (contraction over C=128 on partitions) per 8-row output stripe.
"""

import numpy as np

# problem constants (hardcoded per harness contract)
B, C, H, W = 32, 128, 56, 56
O, KPTS = 128, 9
HK = WK = 7
PAD = 3
HP = H + 2 * PAD          # 62 (padded spatial)
NCORES = 8
BPC = B // NCORES         # 4 images per core
YB = 8                    # output rows per psum tile
NYB = H // YB             # 7
NFREE = YB * W            # 448 moving-operand columns per matmul

_prog_cache = {}


def _build_program(n_img=BPC, n_yb=NYB):
    from contextlib import ExitStack

    import concourse.tile as tile
    from concourse import bacc, mybir

    dt = mybir.dt
    f32 = dt.float32
    f32r = dt.float32r
    Act = mybir.ActivationFunctionType
    Alu = mybir.AluOpType

    nc = bacc.Bacc("TRN2", target_bir_lowering=False, debug=False,
                   num_devices=NCORES)

    x_d = nc.dram_tensor("x", [n_img, C, HP * HP], f32r,
                         kind="ExternalInput").ap()
    wt_d = nc.dram_tensor("wt", [C, KPTS * O], f32, kind="ExternalInput").ap()
    p_d = nc.dram_tensor("p", [C, 2 * KPTS], f32, kind="ExternalInput").ap()
    b_d = nc.dram_tensor("bias", [C, 1], f32, kind="ExternalInput").ap()
    out_d = nc.dram_tensor("out", [n_img, C, H * W], f32,
                           kind="ExternalOutput").ap()

    with tile.TileContext(nc) as tc, ExitStack() as ctx:
        consts = ctx.enter_context(tc.tile_pool(name="consts", bufs=1))
        xpool = ctx.enter_context(tc.tile_pool(name="xpad", bufs=1))
        opool = ctx.enter_context(tc.tile_pool(name="outsb", bufs=4))
        ppool = ctx.enter_context(tc.tile_pool(name="psum", bufs=8,
                                               space="PSUM"))

        p_t = consts.tile([C, 2 * KPTS], f32)       # [c][ph(9) | pw(9)]
        nc.sync.dma_start(p_t[:], p_d[:])
        bias_t = consts.tile([C, 1], f32)
        nc.sync.dma_start(bias_t[:], b_d[:])
        wT = consts.tile([C, KPTS * O], f32)        # [c][k,o]
        nc.sync.dma_start(wT[:], wt_d[:])

        # clip positions to [-3, 3] (both axes at once)
        pc = consts.tile([C, 2 * KPTS], f32)
        nc.vector.tensor_scalar(pc[:], p_t[:], -float(PAD), float(PAD),
                                Alu.max, Alu.min)

        # hat weights on the 7-point grid j:
        #   fhw[c, j, axis*9+k] = relu(1 - |pclip + 3 - j|)
        cbias = consts.tile([C, HK + 1], f32)
        for j in range(HK):
            nc.vector.memset(cbias[:, j:j + 1], float(PAD - j))
        nc.vector.memset(cbias[:, HK:HK + 1], 1.0)
        fhw = consts.tile([C, HK * 2 * KPTS], f32)
        tmp = consts.tile([C, 2 * KPTS], f32)
        for j in range(HK):
            nc.scalar.activation(tmp[:], pc[:], Act.Abs,
                                 bias=cbias[:, j:j + 1], scale=1.0)
            nc.scalar.activation(fhw[:, j * 2 * KPTS:(j + 1) * 2 * KPTS],
                                 tmp[:], Act.Relu, bias=cbias[:, HK:HK + 1],
                                 scale=-1.0)

        # stage A: G[c, k, w*128+o] = wT[c,k,o] * fw[c,k,w]
        # (w-outer + DVE/ACT split so stage B's first half-block only waits
        # on the w<3 slices; ACT does its multiply as Copy-with-scale)
        G = consts.tile([C, KPTS * WK * O], f32)

        def stage_a(w_range):
            for w in w_range:
                for k in range(KPTS):
                    fw_s = fhw[:, w * 2 * KPTS + KPTS + k:
                               w * 2 * KPTS + KPTS + k + 1]
                    g_out = G[:, (k * WK + w) * O:(k * WK + w + 1) * O]
                    w_in = wT[:, k * O:(k + 1) * O]
                    if k % 2 == 0:
                        nc.vector.tensor_scalar(g_out, w_in, fw_s, None,
                                                Alu.mult)
                    else:
                        nc.scalar.mul(g_out, w_in, fw_s)

        # stage B: kern[c, (h*7+w)*128+o] = sum_k fh[c,k,h] * G[c,k,(w,o)]
        # (dense 7x7 kernel in stationary-operand layout, produced in
        # half-blocks in matmul consumption order; f32 accumulator, only the
        # last MAC rounds into the f32r matmul operand)
        kern = consts.tile([C, HK * WK * O], f32r)
        kacc = consts.tile([C, HK * WK * O], f32)
        halves = [(0, 3 * O), (3 * O, WK * O)]

        def stage_b(h, lo, hi):
            for k in range(KPTS):
                fh_s = fhw[:, h * 2 * KPTS + k: h * 2 * KPTS + k + 1]
                ks = kern[:, h * WK * O + lo: h * WK * O + hi]
                ka = kacc[:, h * WK * O + lo: h * WK * O + hi]
                g_s = G[:, k * WK * O + lo: k * WK * O + hi]
                if k == 0:
                    nc.vector.tensor_scalar(ka, g_s, fh_s, None, Alu.mult)
                elif k == KPTS - 1:
                    nc.vector.scalar_tensor_tensor(ks, g_s, fh_s, ka,
                                                   Alu.mult, Alu.add)
                else:
                    nc.vector.scalar_tensor_tensor(ka, g_s, fh_s, ka,
                                                   Alu.mult, Alu.add)

        stage_a(range(0, 3))
        stage_b(0, *halves[0])
        stage_a(range(3, WK))
        stage_b(0, *halves[1])
        for h in range(1, HK):
            for lo, hi in halves:
                stage_b(h, lo, hi)

        xp_tiles = [xpool.tile([C, HP * HP], f32r, tag=f"xp{i}",
                               name=f"xp{i}") for i in range(2)]

        offs = [(dh, dw) for dh in range(HK) for dw in range(WK)]

        def drain(img, yb, ps):
            ob = opool.tile([C, NFREE], f32, name=f"ob{img}_{yb}", tag="ob")
            nc.scalar.activation(ob[:], ps[:], Act.Identity,
                                 bias=bias_t[:, 0:1], scale=1.0)
            nc.sync.dma_start(out_d[img, :, yb * NFREE:(yb + 1) * NFREE],
                              ob[:])

        for img in range(n_img):
            xp = xp_tiles[img % 2]
            nc.sync.dma_start(xp[:], x_d[img])
            xv = xp[:].rearrange("c (r q) -> c r q", q=HP)
            if img == 0:
                # offset-outer: each kern tile is consumed 7x back-to-back,
                # so the PE keeps pace with the (concurrent) kernel build
                pss = [ppool.tile([C, NFREE], f32, name=f"ps0_{yb}", tag="ps")
                       for yb in range(n_yb)]
                for i, (dh, dw) in enumerate(offs):
                    for yb in range(n_yb):
                        rhs = xv[:, yb * YB + dh: yb * YB + dh + YB,
                                 dw: dw + W]
                        nc.tensor.matmul(pss[yb][:],
                                         kern[:, i * O:(i + 1) * O], rhs,
                                         start=(i == 0),
                                         stop=(i == len(offs) - 1),
                                         skip_group_check=True)
                for yb in range(n_yb):
                    drain(img, yb, pss[yb])
            else:
                # stripe-outer: one PSUM bank at a time, rolling drains
                for yb in range(n_yb):
                    ps = ppool.tile([C, NFREE], f32, name=f"ps{img}_{yb}", tag="ps")
                    for i, (dh, dw) in enumerate(offs):
                        rhs = xv[:, yb * YB + dh: yb * YB + dh + YB,
                                 dw: dw + W]
                        nc.tensor.matmul(ps[:], kern[:, i * O:(i + 1) * O],
                                         rhs, start=(i == 0),
                                         stop=(i == len(offs) - 1))
                    drain(img, yb, ps)

    nc.compile()
    return nc


def _get_nc():
    if "nc" not in _prog_cache:
        _prog_cache["nc"] = _build_program()
    return _prog_cache["nc"]


def _prep_in_maps(x, weight, P, bias):
    x = np.asarray(x, dtype=np.float32)
    weight = np.asarray(weight, dtype=np.float32)
    P = np.asarray(P, dtype=np.float32)
    bias = np.asarray(bias, dtype=np.float32)

    xp = np.zeros((B, C, HP, HP), np.float32)
    xp[:, :, PAD:PAD + H, PAD:PAD + W] = x
    xp = xp.reshape(NCORES, BPC, C, HP * HP)
    wt = np.ascontiguousarray(weight.transpose(1, 2, 0)).reshape(C, KPTS * O)
    p2 = np.ascontiguousarray(P.transpose(1, 0, 2)).reshape(C, 2 * KPTS)
    b2 = np.ascontiguousarray(bias.reshape(C, 1))
    return [{"x": np.ascontiguousarray(xp[i]), "wt": wt, "p": p2, "bias": b2}
            for i in range(NCORES)]


def _run(in_maps, trace=False):
    from concourse.bass_utils import run_bass_kernel_spmd
    nc = _get_nc()
    res = run_bass_kernel_spmd(nc, in_maps, list(range(NCORES)), trace=trace)
    out = np.concatenate(
        [np.asarray(res.results[i]["out"]).reshape(BPC, C, H, W)
         for i in range(NCORES)], axis=0)
    return out, res


def kernel(x, weight, P, bias):
    out, _ = _run(_prep_in_maps(x, weight, P, bias), trace=False)
    return out


# revision 17
# speedup vs baseline: 1.0003x; 1.0003x over previous
"""Dcls2d (dilated conv with learnable spacings) on 8 Trainium2 NeuronCores.

Math: kern[o,c,h,w] = sum_k weight[o,c,k] * hat(ph[c,k]-h) * hat(pw[c,k]-w)
      (hat(t) = relu(1-|t|) reproduces the reference's bilinear corner fracs
      bit-exactly), then out = conv2d(x, kern, pad=3) + bias.

Sharding: data-parallel over batch — 4 images per core, weight/P/bias
replicated; the (tiny) kernel construction is redone on every core on the
vector engine, the conv runs on the tensor engine as 49 PSUM-accumulated
matmuls (contraction over C=128 on partitions) per 8-row output stripe.
"""

import numpy as np

# problem constants (hardcoded per harness contract)
B, C, H, W = 32, 128, 56, 56
O, KPTS = 128, 9
HK = WK = 7
PAD = 3
HP = H + 2 * PAD          # 62 (padded spatial)
NCORES = 8
BPC = B // NCORES         # 4 images per core
YB = 8                    # output rows per psum tile
NYB = H // YB             # 7
NFREE = YB * W            # 448 moving-operand columns per matmul

_prog_cache = {}

STATIONARY = "f32r"   # "f32r" | "bf16" — dtype of the conv's stationary operand


def _build_program(n_img=BPC, n_yb=NYB):
    from contextlib import ExitStack

    import concourse.tile as tile
    from concourse import bacc, mybir

    dt = mybir.dt
    f32 = dt.float32
    f32r = dt.float32r
    Act = mybir.ActivationFunctionType
    Alu = mybir.AluOpType

    nc = bacc.Bacc("TRN2", target_bir_lowering=False, debug=False,
                   num_devices=NCORES)

    x_d = nc.dram_tensor("x", [n_img, C, HP * HP], f32r,
                         kind="ExternalInput").ap()
    wt_d = nc.dram_tensor("wt", [C, KPTS * O], f32, kind="ExternalInput").ap()
    p_d = nc.dram_tensor("p", [C, 2 * KPTS], f32, kind="ExternalInput").ap()
    b_d = nc.dram_tensor("bias", [C, 1], f32, kind="ExternalInput").ap()
    out_d = nc.dram_tensor("out", [n_img, C, H * W], f32,
                           kind="ExternalOutput").ap()

    with tile.TileContext(nc) as tc, ExitStack() as ctx:
        consts = ctx.enter_context(tc.tile_pool(name="consts", bufs=1))
        xpool = ctx.enter_context(tc.tile_pool(name="xpad", bufs=1))
        opool = ctx.enter_context(tc.tile_pool(name="outsb", bufs=4))
        ppool = ctx.enter_context(tc.tile_pool(name="psum", bufs=8,
                                               space="PSUM"))

        p_t = consts.tile([C, 2 * KPTS], f32)       # [c][ph(9) | pw(9)]
        nc.sync.dma_start(p_t[:], p_d[:])
        bias_t = consts.tile([C, 1], f32)
        nc.sync.dma_start(bias_t[:], b_d[:])
        wT = consts.tile([C, KPTS * O], f32)        # [c][k,o]
        nc.sync.dma_start(wT[:], wt_d[:])

        # clip positions to [-3, 3] (both axes at once)
        pc = consts.tile([C, 2 * KPTS], f32)
        nc.vector.tensor_scalar(pc[:], p_t[:], -float(PAD), float(PAD),
                                Alu.max, Alu.min)

        # hat weights on the 7-point grid j:
        #   fhw[c, j, axis*9+k] = relu(1 - |pclip + 3 - j|)
        cbias = consts.tile([C, HK + 1], f32)
        for j in range(HK):
            nc.vector.memset(cbias[:, j:j + 1], float(PAD - j))
        nc.vector.memset(cbias[:, HK:HK + 1], 1.0)
        fhw = consts.tile([C, HK * 2 * KPTS], f32)
        tmp = consts.tile([C, 2 * KPTS], f32)
        for j in range(HK):
            nc.scalar.activation(tmp[:], pc[:], Act.Abs,
                                 bias=cbias[:, j:j + 1], scale=1.0)
            nc.scalar.activation(fhw[:, j * 2 * KPTS:(j + 1) * 2 * KPTS],
                                 tmp[:], Act.Relu, bias=cbias[:, HK:HK + 1],
                                 scale=-1.0)

        # stage A: G[c, k, w*128+o] = wT[c,k,o] * fw[c,k,w]
        # (w-outer + DVE/ACT split so stage B's first half-block only waits
        # on the w<3 slices; ACT does its multiply as Copy-with-scale)
        G = consts.tile([C, KPTS * WK * O], f32)

        def stage_a(w_range):
            for w in w_range:
                for k in range(KPTS):
                    fw_s = fhw[:, w * 2 * KPTS + KPTS + k:
                               w * 2 * KPTS + KPTS + k + 1]
                    g_out = G[:, (k * WK + w) * O:(k * WK + w + 1) * O]
                    w_in = wT[:, k * O:(k + 1) * O]
                    if k % 2 == 0:
                        nc.vector.tensor_scalar(g_out, w_in, fw_s, None,
                                                Alu.mult)
                    else:
                        nc.scalar.mul(g_out, w_in, fw_s)

        # stage B: kern[c, (h*7+w)*128+o] = sum_k fh[c,k,h] * G[c,k,(w,o)]
        # (dense 7x7 kernel in stationary-operand layout, produced in
        # half-blocks in matmul consumption order; f32 accumulator, only the
        # last MAC rounds into the f32r matmul operand)
        kern_dt = f32r if STATIONARY == "f32r" else dt.bfloat16
        kern = consts.tile([C, HK * WK * O], kern_dt)
        kacc = consts.tile([C, HK * WK * O], f32)
        halves = [(0, 3 * O), (3 * O, WK * O)]

        def stage_b(h, lo, hi):
            for k in range(KPTS):
                fh_s = fhw[:, h * 2 * KPTS + k: h * 2 * KPTS + k + 1]
                ks = kern[:, h * WK * O + lo: h * WK * O + hi]
                ka = kacc[:, h * WK * O + lo: h * WK * O + hi]
                g_s = G[:, k * WK * O + lo: k * WK * O + hi]
                if k == 0:
                    nc.vector.tensor_scalar(ka, g_s, fh_s, None, Alu.mult)
                elif k == KPTS - 1:
                    nc.vector.scalar_tensor_tensor(ks, g_s, fh_s, ka,
                                                   Alu.mult, Alu.add)
                else:
                    nc.vector.scalar_tensor_tensor(ka, g_s, fh_s, ka,
                                                   Alu.mult, Alu.add)

        stage_a(range(0, 3))
        stage_b(0, *halves[0])
        stage_a(range(3, WK))
        stage_b(0, *halves[1])
        for h in range(1, HK):
            for lo, hi in halves:
                stage_b(h, lo, hi)

        xp_tiles = [xpool.tile([C, HP * HP], f32r, tag=f"xp{i}",
                               name=f"xp{i}") for i in range(2)]

        offs = [(dh, dw) for dh in range(HK) for dw in range(WK)]

        def drain(img, yb, ps):
            ob = opool.tile([C, NFREE], f32, name=f"ob{img}_{yb}", tag="ob")
            nc.scalar.activation(ob[:], ps[:], Act.Identity,
                                 bias=bias_t[:, 0:1], scale=1.0)
            nc.sync.dma_start(out_d[img, :, yb * NFREE:(yb + 1) * NFREE],
                              ob[:])

        for img in range(n_img):
            xp = xp_tiles[img % 2]
            nc.sync.dma_start(xp[:], x_d[img])
            xv = xp[:].rearrange("c (r q) -> c r q", q=HP)
            if img == 0:
                # offset-outer: each kern tile is consumed 7x back-to-back,
                # so the PE keeps pace with the (concurrent) kernel build
                pss = [ppool.tile([C, NFREE], f32, name=f"ps0_{yb}", tag="ps")
                       for yb in range(n_yb)]
                for i, (dh, dw) in enumerate(offs):
                    for yb in range(n_yb):
                        rhs = xv[:, yb * YB + dh: yb * YB + dh + YB,
                                 dw: dw + W]
                        nc.tensor.matmul(pss[yb][:],
                                         kern[:, i * O:(i + 1) * O], rhs,
                                         start=(i == 0),
                                         stop=(i == len(offs) - 1),
                                         skip_group_check=True)
                for yb in range(n_yb):
                    drain(img, yb, pss[yb])
            else:
                # stripe-outer: one PSUM bank at a time, rolling drains
                for yb in range(n_yb):
                    ps = ppool.tile([C, NFREE], f32, name=f"ps{img}_{yb}", tag="ps")
                    for i, (dh, dw) in enumerate(offs):
                        rhs = xv[:, yb * YB + dh: yb * YB + dh + YB,
                                 dw: dw + W]
                        nc.tensor.matmul(ps[:], kern[:, i * O:(i + 1) * O],
                                         rhs, start=(i == 0),
                                         stop=(i == len(offs) - 1))
                    drain(img, yb, ps)

    nc.compile()
    return nc


def _get_nc():
    if "nc" not in _prog_cache:
        _prog_cache["nc"] = _build_program()
    return _prog_cache["nc"]


def _prep_in_maps(x, weight, P, bias):
    x = np.asarray(x, dtype=np.float32)
    weight = np.asarray(weight, dtype=np.float32)
    P = np.asarray(P, dtype=np.float32)
    bias = np.asarray(bias, dtype=np.float32)

    xp = np.zeros((B, C, HP, HP), np.float32)
    xp[:, :, PAD:PAD + H, PAD:PAD + W] = x
    xp = xp.reshape(NCORES, BPC, C, HP * HP)
    wt = np.ascontiguousarray(weight.transpose(1, 2, 0)).reshape(C, KPTS * O)
    p2 = np.ascontiguousarray(P.transpose(1, 0, 2)).reshape(C, 2 * KPTS)
    b2 = np.ascontiguousarray(bias.reshape(C, 1))
    return [{"x": np.ascontiguousarray(xp[i]), "wt": wt, "p": p2, "bias": b2}
            for i in range(NCORES)]


def _run(in_maps, trace=False):
    from concourse.bass_utils import run_bass_kernel_spmd
    nc = _get_nc()
    res = run_bass_kernel_spmd(nc, in_maps, list(range(NCORES)), trace=trace)
    out = np.concatenate(
        [np.asarray(res.results[i]["out"]).reshape(BPC, C, H, W)
         for i in range(NCORES)], axis=0)
    return out, res


def kernel(x, weight, P, bias):
    out, _ = _run(_prep_in_maps(x, weight, P, bias), trace=False)
    return out
